# revision 29
# baseline (speedup 1.0000x reference)
"""BinaryTreeCell (binary tree LSTM cell) TRN2 Bass kernel.

Full-input contract: kernel(**inputs) takes the unsharded numpy inputs of
reference.setup_inputs() and returns (c, h), each [131072, 256] float32.

Strategy
--------
Data-parallel over the node dimension N=131072 across 8 NeuronCores
(16384 nodes/core); all weights replicated.

Matmuls run in fp8 (e4m3) with perf_mode=DoubleRow (2 K-rows per PE cell,
0.5 cycles per output row), with mixed-precision operand splitting to stay
inside the 2e-2 relative-error budget:

    z  = [x, lh, rh]            split as  z ~= z8 + zlo      (both e4m3)
    Ag = [W_g.T; Ul_g.T; Ur_g.T]  split as  A ~= A8 + Alo/32  (both e4m3)

  gate  i          (sigmoid, lowest error sensitivity): 1 term
      pre = z8@A8
  gates lf, rf     (sigmoid, low error sensitivity):    2 terms
      pre = z8@A8 + zlo@A8
  gates u, o       (tanh / sigmoid, high sensitivity):  3 terms
      pre = z8@A8 + zlo@A8 + (z8/32)@(32*Alo)
  The 2^-5 / 2^5 exponent shifts keep the residual weights out of the
  e4m3 denormal range (which otherwise floors the error at ~1e-2).
  Measured end-to-end rel error vs the f32 reference: ~1.7e-2.

On-chip layout is transposed (features on partitions, nodes on free dim).
Per 512-node block and 128-feature half, 33 DoubleRow matmuls accumulate
the five gates into 5 PSUM banks arranged as two 2-bank pairs (i,lf) /
(rf,o) plus one bank for u, so the four sigmoids run as two [128,1024]
activations spanning bank pairs. Gate outputs are bf16; the c/h elementwise chain runs on
VectorE in bf16 (2x DVE mode). c and h are stored once per block as
[128,2,bm] tiles; tanh(c) (paired across both halves), the h muls and the
h store are software-pipelined one block behind so the ACT queue never
waits head-of-line. DMA triggers are spread across SP (z streams, rc),
ACT (lc, weights) and Pool/SWDGE (c,h stores); the first block is 128
nodes so the pipeline fills fast, and the tail is split 256+128 to
shorten the exposed epilogue chain.
"""

import numpy as np
import ml_dtypes

N_TOTAL = 131072
D = 256
CORES = 8
NP_ = N_TOTAL // CORES          # 16384 nodes per core
KD = 3 * D                      # 768 contraction
KC = KD // 128                  # 6 contraction chunks of 128
NBLK_MAIN = 31

E4 = ml_dtypes.float8_e4m3fn
BF = ml_dtypes.bfloat16

_CACHE = {}


def _build_nc(use_bias):
    """Build + compile the per-core Bass program (same NEFF for all cores)."""
    import concourse.bass as bass
    import concourse.tile as tile
    from concourse import bacc, mybir

    f32 = mybir.dt.float32
    bf16 = mybir.dt.bfloat16
    f8 = mybir.dt.float8e4
    AF = mybir.ActivationFunctionType
    PM = mybir.MatmulPerfMode

    nc = bacc.Bacc("TRN2", target_bir_lowering=False, debug=False)

    z8T = nc.dram_tensor("z8T", [KD, NP_], f8, kind="ExternalInput").ap()
    zloT = nc.dram_tensor("zloT", [KD, NP_], f8, kind="ExternalInput").ap()
    zsT = nc.dram_tensor("zsT", [KD, NP_], f8, kind="ExternalInput").ap()
    lcT = nc.dram_tensor("lcT", [D, NP_], bf16, kind="ExternalInput").ap()
    rcT = nc.dram_tensor("rcT", [D, NP_], bf16, kind="ExternalInput").ap()
    A8 = nc.dram_tensor("A8", [KD, 10 * 128], f8, kind="ExternalInput").ap()
    Alo = nc.dram_tensor("Alo", [KD, 4 * 128], f8, kind="ExternalInput").ap()
    if use_bias:
        bias = nc.dram_tensor("bias", [128, 10], f32, kind="ExternalInput").ap()
    cT = nc.dram_tensor("cT", [D, NP_], bf16, kind="ExternalOutput").ap()
    hT = nc.dram_tensor("hT", [D, NP_], bf16, kind="ExternalOutput").ap()

    # node blocks: full 512 blocks (512-wide z slices dodge the <512B DMA
    # descriptor penalty; the 512-wide head block's slow-pstate phase covers
    # the weight/stream fill window)
    blocks = []
    off = 0
    for _ in range(32):
        blocks.append((off, 512))
        off += 512
    assert off == NP_

    with tile.TileContext(nc) as tc:
        with (
            tc.tile_pool(name="wpool", bufs=1) as wpool,
            tc.tile_pool(name="zpool", bufs=3) as zpool,
            tc.tile_pool(name="cpool", bufs=3) as cpool,
            tc.tile_pool(name="gb", bufs=3) as gb,
            tc.tile_pool(name="gpool", bufs=2) as gpool,
            tc.tile_pool(name="tpool", bufs=2) as tpool,
            tc.tile_pool(name="opool", bufs=3) as opool,
            tc.tile_pool(name="psum", bufs=1, space="PSUM") as psum,
            tc.tile_pool(name="psu2", bufs=2, space="PSUM") as psu2,
            tc.tile_pool(name="psf", bufs=1, space="PSUM") as psf_pool,
        ):
            # PE clock-ramp warmup: dummy DoubleRow matmuls on memset data
            # keep the PE continuously busy from ~t0 until the first real
            # operands land, so the 3us pstate ramp completes during the
            # DMA fill instead of taxing the first blocks
            wz = wpool.tile([128, 2, 128], f8, tag="wz")
            nc.gpsimd.memset(wz[:], 0.25)
            ww = wpool.tile([128, 2, 128], f8, tag="ww")
            nc.gpsimd.memset(ww[:], 0.25)
            psf = psf_pool.tile([128, 128], f32, tag="fill")
            for fi in range(80):
                nc.tensor.matmul(
                    psf[:], ww[:], wz[:],
                    start=(fi == 0), stop=(fi == 79),
                    perf_mode=PM.DoubleRow,
                )

            a8_sb = wpool.tile([128, KC, 10 * 128], f8, tag="A8")
            alo_sb = wpool.tile([128, KC, 4 * 128], f8, tag="Alo")
            a8_src = A8.rearrange("(kc p) m -> p kc m", p=128)
            alo_src = Alo.rearrange("(kc p) m -> p kc m", p=128)

            def load_z(m0, bm, split_z8=False):
                tiles = []
                for tag, src in (("z8", z8T), ("zlo", zloT), ("zs", zsT)):
                    t = zpool.tile([128, KC, bm], f8, tag=tag)
                    asrc = src[:, m0:m0 + bm].rearrange(
                        "(kc p) m -> p kc m", p=128)
                    if split_z8 and tag == "z8":
                        nc.sync.dma_start(out=t[:, 0:2, :], in_=asrc[:, 0:2, :])
                        nc.sync.dma_start(out=t[:, 2:, :], in_=asrc[:, 2:, :])
                    else:
                        nc.sync.dma_start(out=t[:], in_=asrc[:])
                    tiles.append(t)
                lc_sb = cpool.tile([128, 2, bm], bf16, tag="lc")
                nc.scalar.dma_start(
                    out=lc_sb[:],
                    in_=lcT[:, m0:m0 + bm].rearrange("(f p) m -> p f m", p=128),
                )
                rc_sb = cpool.tile([128, 2, bm], bf16, tag="rc")
                nc.sync.dma_start(
                    out=rc_sb[:],
                    in_=rcT[:, m0:m0 + bm].rearrange("(f p) m -> p f m", p=128),
                )
                return tiles + [lc_sb, rc_sb]

            # startup order: f0 weights first (longest pole for the first
            # matmul), interleaved with the first z block, then f1 weights.
            # lc/rc of block 0 land late; the deferred epilogue absorbs it.
            nc.scalar.dma_start(out=a8_sb[:, :, 0:640], in_=a8_src[:, :, 0:640])
            ztiles = {0: load_z(*blocks[0], split_z8=True)}
            nc.scalar.dma_start(out=alo_sb[:, :, 0:256], in_=alo_src[:, :, 0:256])
            nc.scalar.dma_start(out=a8_sb[:, :, 640:1280],
                                in_=a8_src[:, :, 640:1280])
            nc.scalar.dma_start(out=alo_sb[:, :, 256:512],
                                in_=alo_src[:, :, 256:512])

            # warm the activation tables (tanh + sigmoid share one set);
            # emitted after the DMA triggers so the ACT sequencer fires those
            # first
            warm = wpool.tile([128, 1], f32, tag="warm")
            nc.gpsimd.memset(warm[:], 0.0)
            warm_o = wpool.tile([128, 1], f32, tag="warm_o")
            nc.scalar.activation(warm_o[:], warm[:], AF.Tanh)
            nc.scalar.activation(warm_o[:], warm[:], AF.Sigmoid)
            if use_bias:
                b_sb = wpool.tile([128, 10], f32, tag="b")
                nc.gpsimd.dma_start(out=b_sb[:], in_=bias[:])

            # deferred epilogue from the previous feature half:
            # (c_ap, gB, f, m0, bm)
            pend = [None]

            def flush_pending():
                if pend[0] is None:
                    return
                c_ap, gBp, fp, m0p, bmp, eng = pend[0]
                pend[0] = None
                tc_t = tpool.tile([128, bmp], bf16, tag="tc")
                nc.scalar.activation(tc_t[:], c_ap, AF.Tanh)
                h_t = opool.tile([128, bmp], bf16, tag="h")
                nc.vector.tensor_mul(h_t[:], gBp[:, 1, :], tc_t[:])
                eng.dma_start(
                    out=hT[fp * 128:(fp + 1) * 128, m0p:m0p + bmp],
                    in_=h_t[:],
                )

            for blk, (m0, bm) in enumerate(blocks):
                tail = blk == len(blocks) - 1
                if blk + 1 < len(blocks):
                    ztiles[blk + 1] = load_z(*blocks[blk + 1])
                z8_sb, zlo_sb, zs_sb, lc_sb, rc_sb = ztiles.pop(blk)

                for f in range(2):
                    # always bank-sized (512 f32) so each gate half owns a
                    # full PSUM bank: accumulation groups of different gates
                    # must not share a bank zero-region
                    pA = psum.tile([128, 2, 512], f32, tag="A")   # i, lf
                    pB = psum.tile([128, 2, 512], f32, tag="B")   # rf, o
                    pU = psu2.tile([128, 512], f32, tag="U")      # u

                    zt = {0: z8_sb, 1: zlo_sb, 2: zs_sb}
                    n_i, n_lf, n_rf = 5 * f + 1, 5 * f + 2, 5 * f + 3
                    n_o, n_u = 5 * f + 4, 5 * f
                    lo_u, lo_o = 2 * f, 2 * f + 1

                    def cols(n):
                        return slice(n * 128, (n + 1) * 128)

                    entries = []

                    def gate(bank, out_ap, n, nterms, lo_n=None,
                             kps_by_term=None):
                        for term in range(nterms):
                            at = a8_sb if term < 2 else alo_sb
                            cn = cols(n) if term < 2 else cols(lo_n)
                            kps = (kps_by_term or {}).get(term, (0, 2, 4))
                            for kp in kps:
                                entries.append((bank, out_ap, at, cn,
                                                zt[term], kp, term))

                    if tail:
                        # tail blocks: u first so the epilogue's ACT chain
                        # starts as early as possible
                        gate("U", pU[:, :bm], n_u, 3, lo_u)
                        gate("I", pA[:, 0, :bm], n_i, 1)         # z8 only
                        gate("LF", pA[:, 1, :bm], n_lf, 2,
                             kps_by_term={1: (2, 4)})            # no x-zlo
                        gate("RF", pB[:, 0, :bm], n_rf, 2)
                        gate("O", pB[:, 1, :bm], n_o, 3, lo_o)
                    else:
                        gate("I", pA[:, 0, :bm], n_i, 1)         # z8 only
                        gate("LF", pA[:, 1, :bm], n_lf, 2,
                             kps_by_term={1: (2, 4)})            # no x-zlo
                        gate("RF", pB[:, 0, :bm], n_rf, 2)
                        gate("O", pB[:, 1, :bm], n_o, 3, lo_o)
                        gate("U", pU[:, :bm], n_u, 3, lo_u)

                    if blk == 0:
                        # interleave by (term, k-pair): the first K-pair of
                        # z8 suffices to start the PE
                        entries.sort(key=lambda e: (e[6], e[5]))
                    elif blk == 1:
                        # interleave by term so early blocks start on z8 alone
                        entries.sort(key=lambda e: e[6])
                    total = {}
                    for e in entries:
                        total[e[0]] = total.get(e[0], 0) + 1
                    seen = {}
                    for bank, out_ap, at, cn, z_t, kp, term in entries:
                        k = seen.get(bank, 0)
                        seen[bank] = k + 1
                        nc.tensor.matmul(
                            out_ap, at[:, kp:kp + 2, cn], z_t[:, kp:kp + 2, :],
                            start=(k == 0), stop=(k == total[bank] - 1),
                            perf_mode=PM.DoubleRow,
                        )

                    gA = gpool.tile([128, 2, bm], bf16, tag="gA")
                    gB = gb.tile([128, 2, bm], bf16, tag="gB")
                    gU = gpool.tile([128, bm], bf16, tag="gU")
                    if tail and not use_bias:
                        nc.scalar.activation(gU[:], pU[:, :bm], AF.Tanh)
                        nc.scalar.activation(gA[:], pA[:, :, :bm], AF.Sigmoid)
                        if f == 0:
                            # f1's sigmoid(rf,o) runs as two 256-wide pieces
                            # in the epilogue below
                            nc.scalar.activation(gB[:], pB[:, :, :bm],
                                                 AF.Sigmoid)
                    elif use_bias:
                        nc.scalar.activation(gA[:, 0, :], pA[:, 0, :bm],
                                             AF.Sigmoid,
                                             bias=b_sb[:, n_i:n_i + 1])
                        nc.scalar.activation(gA[:, 1, :], pA[:, 1, :bm],
                                             AF.Sigmoid,
                                             bias=b_sb[:, n_lf:n_lf + 1])
                        nc.scalar.activation(gB[:, 0, :], pB[:, 0, :bm],
                                             AF.Sigmoid,
                                             bias=b_sb[:, n_lf:n_lf + 1])
                        nc.scalar.activation(gB[:, 1, :], pB[:, 1, :bm],
                                             AF.Sigmoid,
                                             bias=b_sb[:, n_o:n_o + 1])
                        nc.scalar.activation(gU[:], pU[:, :bm], AF.Tanh,
                                             bias=b_sb[:, n_u:n_u + 1])
                    else:
                        nc.scalar.activation(gA[:], pA[:, :, :bm], AF.Sigmoid)
                        nc.scalar.activation(gB[:], pB[:, :, :bm], AF.Sigmoid)
                        nc.scalar.activation(gU[:], pU[:, :bm], AF.Tanh)

                    # previous half's tanh(c), h-mul and h-store, now that
                    # this half's activations are queued (no ACT HOL waits)
                    if not (tail and f == 1):
                        flush_pending()

                    if tail and f == 1:
                        # final half-block: pipeline the epilogue in two
                        # 256-wide pieces so the exposed post-matmul chain is
                        # half length
                        gBps = []
                        for p in range(2):
                            pc = slice(p * 256, (p + 1) * 256)
                            gBp = gb.tile([128, 2, 256], bf16, tag=f"gBp{p}")
                            nc.scalar.activation(gBp[:], pB[:, :, pc],
                                                 AF.Sigmoid)
                            gBps.append(gBp)
                        t2 = tpool.tile([128, bm], bf16, tag="t2")
                        nc.vector.tensor_mul(t2[:], gA[:, 1, :], lc_sb[:, f, :])
                        t1 = tpool.tile([128, bm], bf16, tag="t1")
                        nc.vector.tensor_mul(t1[:], gA[:, 0, :], gU[:])
                        nc.vector.tensor_add(t1[:], t1[:], t2[:])
                        for p in range(2):
                            pc = slice(p * 256, (p + 1) * 256)
                            gBp = gBps[p]
                            t3p = tpool.tile([128, 256], bf16, tag=f"t3p{p}")
                            nc.vector.tensor_mul(t3p[:], gBp[:, 0, :],
                                                 rc_sb[:, f, pc])
                            cp = opool.tile([128, 256], bf16, tag=f"cp{p}")
                            nc.vector.tensor_add(cp[:], t1[:, pc], t3p[:])
                            nc.sync.dma_start(
                                out=cT[f * 128:(f + 1) * 128,
                                       m0 + p * 256:m0 + (p + 1) * 256],
                                in_=cp[:],
                            )
                            tcp = tpool.tile([128, 256], bf16, tag=f"tcp{p}")
                            nc.scalar.activation(tcp[:], cp[:], AF.Tanh)
                            hp = opool.tile([128, 256], bf16, tag=f"hp{p}")
                            nc.vector.tensor_mul(hp[:], gBp[:, 1, :], tcp[:])
                            nc.sync.dma_start(
                                out=hT[f * 128:(f + 1) * 128,
                                       m0 + p * 256:m0 + (p + 1) * 256],
                                in_=hp[:],
                            )
                        flush_pending()
                        continue
                    if tail:
                        # epilogue-critical ordering: everything not needing
                        # gB first, then the sigB-dependent chain
                        t2 = tpool.tile([128, bm], bf16, tag="t2")
                        nc.vector.tensor_mul(t2[:], gA[:, 1, :], lc_sb[:, f, :])
                        t1 = tpool.tile([128, bm], bf16, tag="t1")
                        nc.vector.tensor_mul(t1[:], gA[:, 0, :], gU[:])
                        nc.vector.tensor_add(t1[:], t1[:], t2[:])
                        t3 = tpool.tile([128, bm], bf16, tag="t3")
                        nc.vector.tensor_mul(t3[:], gB[:, 0, :], rc_sb[:, f, :])
                        c_t = opool.tile([128, bm], bf16, tag="c")
                        nc.vector.tensor_add(c_t[:], t1[:], t3[:])
                    else:
                        t2 = tpool.tile([128, bm], bf16, tag="t2")
                        nc.vector.tensor_mul(t2[:], gA[:, 1, :], lc_sb[:, f, :])
                        t3 = tpool.tile([128, bm], bf16, tag="t3")
                        nc.vector.tensor_mul(t3[:], gB[:, 0, :], rc_sb[:, f, :])
                        nc.vector.tensor_add(t2[:], t2[:], t3[:])
                        t1 = tpool.tile([128, bm], bf16, tag="t1")
                        nc.vector.tensor_mul(t1[:], gA[:, 0, :], gU[:])
                        c_t = opool.tile([128, bm], bf16, tag="c")
                        nc.vector.tensor_add(c_t[:], t1[:], t2[:])
                    store_eng = nc.sync if tail else nc.gpsimd
                    store_eng.dma_start(
                        out=cT[f * 128:(f + 1) * 128, m0:m0 + bm], in_=c_t[:]
                    )
                    pend[0] = (c_t[:], gB, f, m0, bm, store_eng)

            flush_pending()

    nc.compile()
    return nc


def _pack_weights(W_cx, W_ox, W_fx, W_ix,
                  U_ilh, U_irh, U_lflh, U_lfrh, U_rflh, U_rfrh,
                  U_ulh, U_urh, U_olh, U_orh):
    """A8 [768, 1280] e4m3: col chunk n = 5*f + g holds Ag[:, f*128:(f+1)*128]
    with Ag = [W_g.T; Ul_g.T; Ur_g.T], gates g ordered (u, i, lf, rf, o).
    Alo [768, 512] e4m3: chunks (2*f + {0:u, 1:o}) hold 32*(Ag - A8) for the
    two 3-term gates."""
    gates = [
        (W_cx, U_ulh, U_urh),    # u
        (W_ix, U_ilh, U_irh),    # i
        (W_fx, U_lflh, U_lfrh),  # lf
        (W_fx, U_rflh, U_rfrh),  # rf
        (W_ox, U_olh, U_orh),    # o
    ]
    A8 = np.empty((KD, 10 * 128), dtype=E4)
    Alo = np.empty((KD, 4 * 128), dtype=E4)
    for g, (W, Ul, Ur) in enumerate(gates):
        Ag = np.concatenate([W.T, Ul.T, Ur.T], axis=0)  # [768, 256] f32
        A8g = Ag.astype(E4)
        for f in range(2):
            A8[:, (5 * f + g) * 128:(5 * f + g + 1) * 128] = \
                A8g[:, f * 128:(f + 1) * 128]
        if g in (0, 4):
            res = (32.0 * (Ag - A8g.astype(np.float32))).astype(E4)
            n0 = 0 if g == 0 else 1
            for f in range(2):
                Alo[:, (2 * f + n0) * 128:(2 * f + n0 + 1) * 128] = \
                    res[:, f * 128:(f + 1) * 128]
    return np.ascontiguousarray(A8), np.ascontiguousarray(Alo)


def kernel(x, lc, lh, rc, rh,
           W_cx, b_cx, W_ox, b_ox, W_fx, b_fx, W_ix, b_ix,
           U_ilh, U_irh, U_lflh, U_lfrh, U_rflh, U_rfrh,
           U_ulh, U_urh, U_olh, U_orh):
    from concourse.bass_utils import run_bass_kernel_spmd

    x = np.asarray(x, dtype=np.float32)
    lc = np.asarray(lc, dtype=np.float32)
    lh = np.asarray(lh, dtype=np.float32)
    rc = np.asarray(rc, dtype=np.float32)
    rh = np.asarray(rh, dtype=np.float32)

    A8, Alo = _pack_weights(
        np.asarray(W_cx, np.float32), np.asarray(W_ox, np.float32),
        np.asarray(W_fx, np.float32), np.asarray(W_ix, np.float32),
        np.asarray(U_ilh, np.float32), np.asarray(U_irh, np.float32),
        np.asarray(U_lflh, np.float32), np.asarray(U_lfrh, np.float32),
        np.asarray(U_rflh, np.float32), np.asarray(U_rfrh, np.float32),
        np.asarray(U_ulh, np.float32), np.asarray(U_urh, np.float32),
        np.asarray(U_olh, np.float32), np.asarray(U_orh, np.float32),
    )
    biases = [np.asarray(b, np.float32) for b in (b_cx, b_ix, b_fx, b_ox)]
    use_bias = any(np.any(b) for b in biases)
    bias_pack = None
    if use_bias:
        b_cx, b_ix, b_fx, b_ox = biases
        per_gate = [b_cx, b_ix, b_fx, b_fx, b_ox]  # u, i, lf, rf, o
        bias_pack = np.empty((128, 10), dtype=np.float32)
        for g in range(5):
            for f in range(2):
                bias_pack[:, 5 * f + g] = per_gate[g][f * 128:(f + 1) * 128]

    # fp8 split of the streamed operands (e4m3 hi + e4m3 lo + 2^-5-scaled hi)
    def split(a):
        hi = a.astype(E4)
        hif = hi.astype(np.float32)
        lo = (a - hif).astype(E4)
        sc = (hif * (1.0 / 32.0)).astype(E4)
        return hi, lo, sc

    x8, xlo, xs = split(x)
    l8, llo, ls = split(lh)
    r8, rlo, rs = split(rh)
    lcb = lc.astype(BF)
    rcb = rc.astype(BF)

    key = ("nc", use_bias)
    if key not in _CACHE:
        _CACHE[key] = _build_nc(use_bias)
    nc = _CACHE[key]

    def zstack(a, b, c, sl):
        z = np.empty((KD, NP_), dtype=E4)
        z[0:D] = a[sl].T
        z[D:2 * D] = b[sl].T
        z[2 * D:3 * D] = c[sl].T
        return z

    in_maps = []
    for ci in range(CORES):
        sl = slice(ci * NP_, (ci + 1) * NP_)
        m = {
            "z8T": zstack(x8, l8, r8, sl),
            "zloT": zstack(xlo, llo, rlo, sl),
            "zsT": zstack(xs, ls, rs, sl),
            "lcT": np.ascontiguousarray(lcb[sl].T),
            "rcT": np.ascontiguousarray(rcb[sl].T),
            "A8": A8,
            "Alo": Alo,
        }
        if use_bias:
            m["bias"] = bias_pack
        in_maps.append(m)

    import time as _time
    t0 = _time.time()
    res = None
    for attempt, backoff_s in ((0, 15), (1, 45), (2, None)):
        try:
            res = run_bass_kernel_spmd(nc, in_maps, core_ids=list(range(CORES)))
            break
        except Exception:
            # transient device wedge (e.g. NRT_EXEC_UNIT_UNRECOVERABLE):
            # back off and retry; re-raise on the final attempt
            if backoff_s is None:
                raise
            _time.sleep(backoff_s)
    t1 = _time.time()
    _CACHE["last_wall_s"] = t1 - t0
    _CACHE["last_exec_ns"] = res.exec_time_ns
    _CACHE["nc"] = nc

    c_out = np.empty((N_TOTAL, D), dtype=np.float32)
    h_out = np.empty((N_TOTAL, D), dtype=np.float32)
    for ci in range(CORES):
        sl = slice(ci * NP_, (ci + 1) * NP_)
        c_out[sl] = np.asarray(res.results[ci]["cT"]).astype(np.float32).T
        h_out[sl] = np.asarray(res.results[ci]["hT"]).astype(np.float32).T
    return c_out, h_out


# revision 30
# speedup vs baseline: 1.0015x; 1.0015x over previous
"""BinaryTreeCell (binary tree LSTM cell) TRN2 Bass kernel.

Full-input contract: kernel(**inputs) takes the unsharded numpy inputs of
reference.setup_inputs() and returns (c, h), each [131072, 256] float32.

Strategy
--------
Data-parallel over the node dimension N=131072 across 8 NeuronCores
(16384 nodes/core); all weights replicated.

Matmuls run in fp8 (e4m3) with perf_mode=DoubleRow (2 K-rows per PE cell,
0.5 cycles per output row), with mixed-precision operand splitting to stay
inside the 2e-2 relative-error budget:

    z  = [x, lh, rh]            split as  z ~= z8 + zlo      (both e4m3)
    Ag = [W_g.T; Ul_g.T; Ur_g.T]  split as  A ~= A8 + Alo/32  (both e4m3)

  gate  i          (sigmoid, lowest error sensitivity): 1 term
      pre = z8@A8
  gates lf, rf     (sigmoid, low error sensitivity):    2 terms
      pre = z8@A8 + zlo@A8
  gates u, o       (tanh / sigmoid, high sensitivity):  3 terms
      pre = z8@A8 + zlo@A8 + (z8/32)@(32*Alo)
  The 2^-5 / 2^5 exponent shifts keep the residual weights out of the
  e4m3 denormal range (which otherwise floors the error at ~1e-2).
  Measured end-to-end rel error vs the f32 reference: ~1.7e-2.

On-chip layout is transposed (features on partitions, nodes on free dim).
Per 512-node block and 128-feature half, 33 DoubleRow matmuls accumulate
the five gates into 5 PSUM banks arranged as two 2-bank pairs (i,lf) /
(rf,o) plus one bank for u, so the four sigmoids run as two [128,1024]
activations spanning bank pairs. Gate outputs are bf16; the c/h elementwise chain runs on
VectorE in bf16 (2x DVE mode). c and h are stored once per block as
[128,2,bm] tiles; tanh(c) (paired across both halves), the h muls and the
h store are software-pipelined one block behind so the ACT queue never
waits head-of-line. DMA triggers are spread across SP (z streams, rc),
ACT (lc, weights) and Pool/SWDGE (c,h stores); the first block is 128
nodes so the pipeline fills fast, and the tail is split 256+128 to
shorten the exposed epilogue chain.
"""

import numpy as np
import ml_dtypes

N_TOTAL = 131072
D = 256
CORES = 8
NP_ = N_TOTAL // CORES          # 16384 nodes per core
KD = 3 * D                      # 768 contraction
KC = KD // 128                  # 6 contraction chunks of 128
NBLK_MAIN = 31

E4 = ml_dtypes.float8_e4m3fn
BF = ml_dtypes.bfloat16

_CACHE = {}


def _build_nc(use_bias):
    """Build + compile the per-core Bass program (same NEFF for all cores)."""
    import concourse.bass as bass
    import concourse.tile as tile
    from concourse import bacc, mybir

    f32 = mybir.dt.float32
    bf16 = mybir.dt.bfloat16
    f8 = mybir.dt.float8e4
    AF = mybir.ActivationFunctionType
    PM = mybir.MatmulPerfMode

    nc = bacc.Bacc("TRN2", target_bir_lowering=False, debug=False)

    z8T = nc.dram_tensor("z8T", [KD, NP_], f8, kind="ExternalInput").ap()
    zloT = nc.dram_tensor("zloT", [KD, NP_], f8, kind="ExternalInput").ap()
    zsT = nc.dram_tensor("zsT", [KD, NP_], f8, kind="ExternalInput").ap()
    lcT = nc.dram_tensor("lcT", [D, NP_], bf16, kind="ExternalInput").ap()
    rcT = nc.dram_tensor("rcT", [D, NP_], bf16, kind="ExternalInput").ap()
    A8 = nc.dram_tensor("A8", [KD, 10 * 128], f8, kind="ExternalInput").ap()
    Alo = nc.dram_tensor("Alo", [KD, 4 * 128], f8, kind="ExternalInput").ap()
    if use_bias:
        bias = nc.dram_tensor("bias", [128, 10], f32, kind="ExternalInput").ap()
    cT = nc.dram_tensor("cT", [D, NP_], bf16, kind="ExternalOutput").ap()
    hT = nc.dram_tensor("hT", [D, NP_], bf16, kind="ExternalOutput").ap()

    # node blocks: full 512 blocks (512-wide z slices dodge the <512B DMA
    # descriptor penalty; the 512-wide head block's slow-pstate phase covers
    # the weight/stream fill window)
    blocks = []
    off = 0
    for _ in range(32):
        blocks.append((off, 512))
        off += 512
    assert off == NP_

    with tile.TileContext(nc) as tc:
        with (
            tc.tile_pool(name="wpool", bufs=1) as wpool,
            tc.tile_pool(name="zpool", bufs=3) as zpool,
            tc.tile_pool(name="cpool", bufs=3) as cpool,
            tc.tile_pool(name="gb", bufs=3) as gb,
            tc.tile_pool(name="gpool", bufs=2) as gpool,
            tc.tile_pool(name="tpool", bufs=2) as tpool,
            tc.tile_pool(name="opool", bufs=3) as opool,
            tc.tile_pool(name="psum", bufs=1, space="PSUM") as psum,
            tc.tile_pool(name="psu2", bufs=2, space="PSUM") as psu2,
            tc.tile_pool(name="psf", bufs=1, space="PSUM") as psf_pool,
        ):
            # PE clock-ramp warmup: dummy DoubleRow matmuls on memset data
            # keep the PE continuously busy from ~t0 until the first real
            # operands land, so the 3us pstate ramp completes during the
            # DMA fill instead of taxing the first blocks
            wz = wpool.tile([128, 2, 128], f8, tag="wz")
            nc.gpsimd.memset(wz[:], 0.25)
            ww = wpool.tile([128, 2, 128], f8, tag="ww")
            nc.gpsimd.memset(ww[:], 0.25)
            psf = psf_pool.tile([128, 128], f32, tag="fill")
            for fi in range(80):
                nc.tensor.matmul(
                    psf[:], ww[:], wz[:],
                    start=(fi == 0), stop=(fi == 79),
                    perf_mode=PM.DoubleRow,
                )

            a8_sb = wpool.tile([128, KC, 10 * 128], f8, tag="A8")
            alo_sb = wpool.tile([128, KC, 4 * 128], f8, tag="Alo")
            a8_src = A8.rearrange("(kc p) m -> p kc m", p=128)
            alo_src = Alo.rearrange("(kc p) m -> p kc m", p=128)

            def load_z(m0, bm, split_z8=False):
                tiles = []
                for tag, src in (("z8", z8T), ("zlo", zloT), ("zs", zsT)):
                    t = zpool.tile([128, KC, bm], f8, tag=tag)
                    asrc = src[:, m0:m0 + bm].rearrange(
                        "(kc p) m -> p kc m", p=128)
                    if split_z8 and tag == "z8":
                        nc.sync.dma_start(out=t[:, 0:2, :], in_=asrc[:, 0:2, :])
                        nc.sync.dma_start(out=t[:, 2:, :], in_=asrc[:, 2:, :])
                    else:
                        nc.sync.dma_start(out=t[:], in_=asrc[:])
                    tiles.append(t)
                lc_sb = cpool.tile([128, 2, bm], bf16, tag="lc")
                nc.scalar.dma_start(
                    out=lc_sb[:],
                    in_=lcT[:, m0:m0 + bm].rearrange("(f p) m -> p f m", p=128),
                )
                rc_sb = cpool.tile([128, 2, bm], bf16, tag="rc")
                nc.sync.dma_start(
                    out=rc_sb[:],
                    in_=rcT[:, m0:m0 + bm].rearrange("(f p) m -> p f m", p=128),
                )
                return tiles + [lc_sb, rc_sb]

            # startup order: f0 weights first (longest pole for the first
            # matmul), interleaved with the first z block, then f1 weights.
            # lc/rc of block 0 land late; the deferred epilogue absorbs it.
            nc.scalar.dma_start(out=a8_sb[:, :, 0:640], in_=a8_src[:, :, 0:640])
            ztiles = {0: load_z(*blocks[0], split_z8=True)}
            nc.scalar.dma_start(out=alo_sb[:, :, 0:256], in_=alo_src[:, :, 0:256])
            nc.scalar.dma_start(out=a8_sb[:, :, 640:1280],
                                in_=a8_src[:, :, 640:1280])
            nc.scalar.dma_start(out=alo_sb[:, :, 256:512],
                                in_=alo_src[:, :, 256:512])

            # warm the activation tables (tanh + sigmoid share one set);
            # emitted after the DMA triggers so the ACT sequencer fires those
            # first
            warm = wpool.tile([128, 1], f32, tag="warm")
            nc.gpsimd.memset(warm[:], 0.0)
            warm_o = wpool.tile([128, 1], f32, tag="warm_o")
            nc.scalar.activation(warm_o[:], warm[:], AF.Tanh)
            nc.scalar.activation(warm_o[:], warm[:], AF.Sigmoid)
            if use_bias:
                b_sb = wpool.tile([128, 10], f32, tag="b")
                nc.gpsimd.dma_start(out=b_sb[:], in_=bias[:])

            # deferred epilogue from the previous feature half:
            # (c_ap, gB, f, m0, bm)
            pend = [None]

            def flush_pending():
                if pend[0] is None:
                    return
                c_ap, gBp, fp, m0p, bmp, eng = pend[0]
                pend[0] = None
                tc_t = tpool.tile([128, bmp], bf16, tag="tc")
                nc.scalar.activation(tc_t[:], c_ap, AF.Tanh)
                h_t = opool.tile([128, bmp], bf16, tag="h")
                nc.vector.tensor_mul(h_t[:], gBp[:, 1, :], tc_t[:])
                eng.dma_start(
                    out=hT[fp * 128:(fp + 1) * 128, m0p:m0p + bmp],
                    in_=h_t[:],
                )

            for blk, (m0, bm) in enumerate(blocks):
                tail = blk == len(blocks) - 1
                if blk + 1 < len(blocks):
                    ztiles[blk + 1] = load_z(*blocks[blk + 1])
                z8_sb, zlo_sb, zs_sb, lc_sb, rc_sb = ztiles.pop(blk)

                for f in range(2):
                    # always bank-sized (512 f32) so each gate half owns a
                    # full PSUM bank: accumulation groups of different gates
                    # must not share a bank zero-region
                    pA = psum.tile([128, 2, 512], f32, tag="A")   # i, lf
                    pB = psum.tile([128, 2, 512], f32, tag="B")   # rf, o
                    pU = psu2.tile([128, 512], f32, tag="U")      # u

                    zt = {0: z8_sb, 1: zlo_sb, 2: zs_sb}
                    n_i, n_lf, n_rf = 5 * f + 1, 5 * f + 2, 5 * f + 3
                    n_o, n_u = 5 * f + 4, 5 * f
                    lo_u, lo_o = 2 * f, 2 * f + 1

                    def cols(n):
                        return slice(n * 128, (n + 1) * 128)

                    entries = []

                    def gate(bank, out_ap, n, nterms, lo_n=None,
                             kps_by_term=None):
                        for term in range(nterms):
                            at = a8_sb if term < 2 else alo_sb
                            cn = cols(n) if term < 2 else cols(lo_n)
                            kps = (kps_by_term or {}).get(term, (0, 2, 4))
                            for kp in kps:
                                entries.append((bank, out_ap, at, cn,
                                                zt[term], kp, term))

                    if tail:
                        # tail blocks: u first so the epilogue's ACT chain
                        # starts as early as possible
                        gate("U", pU[:, :bm], n_u, 3, lo_u)
                        gate("I", pA[:, 0, :bm], n_i, 1)         # z8 only
                        gate("LF", pA[:, 1, :bm], n_lf, 2,
                             kps_by_term={1: (2, 4)})            # no x-zlo
                        gate("RF", pB[:, 0, :bm], n_rf, 2)
                        gate("O", pB[:, 1, :bm], n_o, 3, lo_o)
                    else:
                        gate("I", pA[:, 0, :bm], n_i, 1)         # z8 only
                        gate("LF", pA[:, 1, :bm], n_lf, 2,
                             kps_by_term={1: (2, 4)})            # no x-zlo
                        gate("RF", pB[:, 0, :bm], n_rf, 2)
                        gate("O", pB[:, 1, :bm], n_o, 3, lo_o)
                        gate("U", pU[:, :bm], n_u, 3, lo_u)

                    if blk == 0:
                        # interleave by (term, k-pair): the first K-pair of
                        # z8 suffices to start the PE
                        entries.sort(key=lambda e: (e[6], e[5]))
                    elif blk == 1:
                        # interleave by term so early blocks start on z8 alone
                        entries.sort(key=lambda e: e[6])
                    total = {}
                    for e in entries:
                        total[e[0]] = total.get(e[0], 0) + 1
                    seen = {}
                    for bank, out_ap, at, cn, z_t, kp, term in entries:
                        k = seen.get(bank, 0)
                        seen[bank] = k + 1
                        nc.tensor.matmul(
                            out_ap, at[:, kp:kp + 2, cn], z_t[:, kp:kp + 2, :],
                            start=(k == 0), stop=(k == total[bank] - 1),
                            perf_mode=PM.DoubleRow,
                        )

                    gA = gpool.tile([128, 2, bm], bf16, tag="gA")
                    gB = gb.tile([128, 2, bm], bf16, tag="gB")
                    gU = gpool.tile([128, bm], bf16, tag="gU")
                    if tail and not use_bias:
                        nc.scalar.activation(gU[:], pU[:, :bm], AF.Tanh)
                        nc.scalar.activation(gA[:], pA[:, :, :bm], AF.Sigmoid)
                        if f == 0:
                            # f1's sigmoid(rf,o) runs as two 256-wide pieces
                            # in the epilogue below
                            nc.scalar.activation(gB[:], pB[:, :, :bm],
                                                 AF.Sigmoid)
                    elif use_bias:
                        nc.scalar.activation(gA[:, 0, :], pA[:, 0, :bm],
                                             AF.Sigmoid,
                                             bias=b_sb[:, n_i:n_i + 1])
                        nc.scalar.activation(gA[:, 1, :], pA[:, 1, :bm],
                                             AF.Sigmoid,
                                             bias=b_sb[:, n_lf:n_lf + 1])
                        nc.scalar.activation(gB[:, 0, :], pB[:, 0, :bm],
                                             AF.Sigmoid,
                                             bias=b_sb[:, n_lf:n_lf + 1])
                        nc.scalar.activation(gB[:, 1, :], pB[:, 1, :bm],
                                             AF.Sigmoid,
                                             bias=b_sb[:, n_o:n_o + 1])
                        nc.scalar.activation(gU[:], pU[:, :bm], AF.Tanh,
                                             bias=b_sb[:, n_u:n_u + 1])
                    else:
                        nc.scalar.activation(gA[:], pA[:, :, :bm], AF.Sigmoid)
                        nc.scalar.activation(gB[:], pB[:, :, :bm], AF.Sigmoid)
                        nc.scalar.activation(gU[:], pU[:, :bm], AF.Tanh)

                    # previous half's tanh(c), h-mul and h-store, now that
                    # this half's activations are queued (no ACT HOL waits)
                    flush_pending()

                    if tail and f == 1:
                        # final half-block: pipeline the epilogue in two
                        # 256-wide pieces so the exposed post-matmul chain is
                        # half length
                        t2 = tpool.tile([128, bm], bf16, tag="t2")
                        nc.vector.tensor_mul(t2[:], gA[:, 1, :], lc_sb[:, f, :])
                        t1 = tpool.tile([128, bm], bf16, tag="t1")
                        nc.vector.tensor_mul(t1[:], gA[:, 0, :], gU[:])
                        nc.vector.tensor_add(t1[:], t1[:], t2[:])
                        for p in range(2):
                            pc = slice(p * 256, (p + 1) * 256)
                            gBp = gb.tile([128, 2, 256], bf16, tag=f"gBp{p}")
                            nc.scalar.activation(gBp[:], pB[:, :, pc],
                                                 AF.Sigmoid)
                            t3p = tpool.tile([128, 256], bf16, tag=f"t3p{p}")
                            nc.vector.tensor_mul(t3p[:], gBp[:, 0, :],
                                                 rc_sb[:, f, pc])
                            cp = opool.tile([128, 256], bf16, tag=f"cp{p}")
                            nc.vector.tensor_add(cp[:], t1[:, pc], t3p[:])
                            nc.sync.dma_start(
                                out=cT[f * 128:(f + 1) * 128,
                                       m0 + p * 256:m0 + (p + 1) * 256],
                                in_=cp[:],
                            )
                            tcp = tpool.tile([128, 256], bf16, tag=f"tcp{p}")
                            nc.scalar.activation(tcp[:], cp[:], AF.Tanh)
                            hp = opool.tile([128, 256], bf16, tag=f"hp{p}")
                            nc.vector.tensor_mul(hp[:], gBp[:, 1, :], tcp[:])
                            nc.sync.dma_start(
                                out=hT[f * 128:(f + 1) * 128,
                                       m0 + p * 256:m0 + (p + 1) * 256],
                                in_=hp[:],
                            )
                        flush_pending()
                        continue
                    if tail:
                        # epilogue-critical ordering: everything not needing
                        # gB first, then the sigB-dependent chain
                        t2 = tpool.tile([128, bm], bf16, tag="t2")
                        nc.vector.tensor_mul(t2[:], gA[:, 1, :], lc_sb[:, f, :])
                        t1 = tpool.tile([128, bm], bf16, tag="t1")
                        nc.vector.tensor_mul(t1[:], gA[:, 0, :], gU[:])
                        nc.vector.tensor_add(t1[:], t1[:], t2[:])
                        t3 = tpool.tile([128, bm], bf16, tag="t3")
                        nc.vector.tensor_mul(t3[:], gB[:, 0, :], rc_sb[:, f, :])
                        c_t = opool.tile([128, bm], bf16, tag="c")
                        nc.vector.tensor_add(c_t[:], t1[:], t3[:])
                    else:
                        t2 = tpool.tile([128, bm], bf16, tag="t2")
                        nc.vector.tensor_mul(t2[:], gA[:, 1, :], lc_sb[:, f, :])
                        t3 = tpool.tile([128, bm], bf16, tag="t3")
                        nc.vector.tensor_mul(t3[:], gB[:, 0, :], rc_sb[:, f, :])
                        nc.vector.tensor_add(t2[:], t2[:], t3[:])
                        t1 = tpool.tile([128, bm], bf16, tag="t1")
                        nc.vector.tensor_mul(t1[:], gA[:, 0, :], gU[:])
                        c_t = opool.tile([128, bm], bf16, tag="c")
                        nc.vector.tensor_add(c_t[:], t1[:], t2[:])
                    store_eng = nc.sync if tail else nc.gpsimd
                    store_eng.dma_start(
                        out=cT[f * 128:(f + 1) * 128, m0:m0 + bm], in_=c_t[:]
                    )
                    pend[0] = (c_t[:], gB, f, m0, bm, store_eng)

            flush_pending()

    nc.compile()
    return nc


def _pack_weights(W_cx, W_ox, W_fx, W_ix,
                  U_ilh, U_irh, U_lflh, U_lfrh, U_rflh, U_rfrh,
                  U_ulh, U_urh, U_olh, U_orh):
    """A8 [768, 1280] e4m3: col chunk n = 5*f + g holds Ag[:, f*128:(f+1)*128]
    with Ag = [W_g.T; Ul_g.T; Ur_g.T], gates g ordered (u, i, lf, rf, o).
    Alo [768, 512] e4m3: chunks (2*f + {0:u, 1:o}) hold 32*(Ag - A8) for the
    two 3-term gates."""
    gates = [
        (W_cx, U_ulh, U_urh),    # u
        (W_ix, U_ilh, U_irh),    # i
        (W_fx, U_lflh, U_lfrh),  # lf
        (W_fx, U_rflh, U_rfrh),  # rf
        (W_ox, U_olh, U_orh),    # o
    ]
    A8 = np.empty((KD, 10 * 128), dtype=E4)
    Alo = np.empty((KD, 4 * 128), dtype=E4)
    for g, (W, Ul, Ur) in enumerate(gates):
        Ag = np.concatenate([W.T, Ul.T, Ur.T], axis=0)  # [768, 256] f32
        A8g = Ag.astype(E4)
        for f in range(2):
            A8[:, (5 * f + g) * 128:(5 * f + g + 1) * 128] = \
                A8g[:, f * 128:(f + 1) * 128]
        if g in (0, 4):
            res = (32.0 * (Ag - A8g.astype(np.float32))).astype(E4)
            n0 = 0 if g == 0 else 1
            for f in range(2):
                Alo[:, (2 * f + n0) * 128:(2 * f + n0 + 1) * 128] = \
                    res[:, f * 128:(f + 1) * 128]
    return np.ascontiguousarray(A8), np.ascontiguousarray(Alo)


def kernel(x, lc, lh, rc, rh,
           W_cx, b_cx, W_ox, b_ox, W_fx, b_fx, W_ix, b_ix,
           U_ilh, U_irh, U_lflh, U_lfrh, U_rflh, U_rfrh,
           U_ulh, U_urh, U_olh, U_orh):
    from concourse.bass_utils import run_bass_kernel_spmd

    x = np.asarray(x, dtype=np.float32)
    lc = np.asarray(lc, dtype=np.float32)
    lh = np.asarray(lh, dtype=np.float32)
    rc = np.asarray(rc, dtype=np.float32)
    rh = np.asarray(rh, dtype=np.float32)

    A8, Alo = _pack_weights(
        np.asarray(W_cx, np.float32), np.asarray(W_ox, np.float32),
        np.asarray(W_fx, np.float32), np.asarray(W_ix, np.float32),
        np.asarray(U_ilh, np.float32), np.asarray(U_irh, np.float32),
        np.asarray(U_lflh, np.float32), np.asarray(U_lfrh, np.float32),
        np.asarray(U_rflh, np.float32), np.asarray(U_rfrh, np.float32),
        np.asarray(U_ulh, np.float32), np.asarray(U_urh, np.float32),
        np.asarray(U_olh, np.float32), np.asarray(U_orh, np.float32),
    )
    biases = [np.asarray(b, np.float32) for b in (b_cx, b_ix, b_fx, b_ox)]
    use_bias = any(np.any(b) for b in biases)
    bias_pack = None
    if use_bias:
        b_cx, b_ix, b_fx, b_ox = biases
        per_gate = [b_cx, b_ix, b_fx, b_fx, b_ox]  # u, i, lf, rf, o
        bias_pack = np.empty((128, 10), dtype=np.float32)
        for g in range(5):
            for f in range(2):
                bias_pack[:, 5 * f + g] = per_gate[g][f * 128:(f + 1) * 128]

    # fp8 split of the streamed operands (e4m3 hi + e4m3 lo + 2^-5-scaled hi)
    def split(a):
        hi = a.astype(E4)
        hif = hi.astype(np.float32)
        lo = (a - hif).astype(E4)
        sc = (hif * (1.0 / 32.0)).astype(E4)
        return hi, lo, sc

    x8, xlo, xs = split(x)
    l8, llo, ls = split(lh)
    r8, rlo, rs = split(rh)
    lcb = lc.astype(BF)
    rcb = rc.astype(BF)

    key = ("nc", use_bias)
    if key not in _CACHE:
        _CACHE[key] = _build_nc(use_bias)
    nc = _CACHE[key]

    def zstack(a, b, c, sl):
        z = np.empty((KD, NP_), dtype=E4)
        z[0:D] = a[sl].T
        z[D:2 * D] = b[sl].T
        z[2 * D:3 * D] = c[sl].T
        return z

    in_maps = []
    for ci in range(CORES):
        sl = slice(ci * NP_, (ci + 1) * NP_)
        m = {
            "z8T": zstack(x8, l8, r8, sl),
            "zloT": zstack(xlo, llo, rlo, sl),
            "zsT": zstack(xs, ls, rs, sl),
            "lcT": np.ascontiguousarray(lcb[sl].T),
            "rcT": np.ascontiguousarray(rcb[sl].T),
            "A8": A8,
            "Alo": Alo,
        }
        if use_bias:
            m["bias"] = bias_pack
        in_maps.append(m)

    import time as _time
    t0 = _time.time()
    res = None
    for attempt, backoff_s in ((0, 15), (1, 45), (2, None)):
        try:
            res = run_bass_kernel_spmd(nc, in_maps, core_ids=list(range(CORES)))
            break
        except Exception:
            # transient device wedge (e.g. NRT_EXEC_UNIT_UNRECOVERABLE):
            # back off and retry; re-raise on the final attempt
            if backoff_s is None:
                raise
            _time.sleep(backoff_s)
    t1 = _time.time()
    _CACHE["last_wall_s"] = t1 - t0
    _CACHE["last_exec_ns"] = res.exec_time_ns
    _CACHE["nc"] = nc

    c_out = np.empty((N_TOTAL, D), dtype=np.float32)
    h_out = np.empty((N_TOTAL, D), dtype=np.float32)
    for ci in range(CORES):
        sl = slice(ci * NP_, (ci + 1) * NP_)
        c_out[sl] = np.asarray(res.results[ci]["cT"]).astype(np.float32).T
        h_out[sl] = np.asarray(res.results[ci]["hT"]).astype(np.float32).T
    return c_out, h_out
